# revision 4
# baseline (speedup 1.0000x reference)
"""Bahdanau additive attention on 8 trn2 NeuronCores.

Computation (per batch b):
    eh = enc[b] @ Wh + bh                    # [S, A]
    dh = dec[b] @ Ws + bs                    # [T, A]
    scores[t, s] = Wv . tanh(eh[s] + dh[t])  (+ bv, dropped: softmax-invariant)
    out[t, :] = softmax(scores[t, :])

Sharding: core c handles batch b = c//2 and decoder rows t in
[128*(c%2), 128*(c%2)+128).  Weights replicated; no cross-core comm.

Per-core kernel layout: A (=256) on partitions in two 128-chunks.
ScalarE activation computes tanh(ehT[a, s] + dhT[a, t]) with the
broadcast-add fused in via the per-partition bias operand.  The
weighted reduction over A is a TensorE matmul with lhsT = Wv [128, 1]
writing scores rows [1, S] into PSUM; softmax is batched on the
[128, 1024] scores tile at the end.
"""

import sys

import numpy as np

sys.path.insert(0, "/opt/trn_rl_repo")

import concourse.bass as bass
import concourse.bacc as bacc
import concourse.tile as tile
from concourse import mybir
from concourse.bass_utils import run_bass_kernel_spmd

B, S, T, H, A = 4, 1024, 256, 512, 256
NCORES = 8
TCORE = (B * T) // NCORES  # 128 decoder rows per core
F32 = mybir.dt.float32
P = 128
KH = H // P  # 4 contraction chunks for the projections
JA = A // P  # 2 partition chunks of the attention dim
NSH = S // 512  # 2 matmul free-dim slices of S


def build_bass() -> bass.Bass:
    nc = bacc.Bacc()
    encT = nc.declare_dram_parameter("encT", [H, S], F32, isOutput=False)
    decT = nc.declare_dram_parameter("decT", [H, TCORE], F32, isOutput=False)
    wh = nc.declare_dram_parameter("wh", [H, A], F32, isOutput=False)
    ws = nc.declare_dram_parameter("ws", [H, A], F32, isOutput=False)
    bsum = nc.declare_dram_parameter("bsum", [A, 1], F32, isOutput=False)
    wv = nc.declare_dram_parameter("wv", [A, 1], F32, isOutput=False)
    out = nc.declare_dram_parameter("out", [TCORE, S], F32, isOutput=True)

    with tile.TileContext(nc) as tc:
        with tc.tile_pool(name="const", bufs=1) as cpool:
            encT_sb = []
            decT_sb = []
            wh_sb = []
            ws_sb = []
            for k in range(KH):
                te = cpool.tile([P, S], F32, tag=f"encT{k}", name=f"encT{k}")
                nc.sync.dma_start(te[:], encT[k * P : (k + 1) * P, :])
                encT_sb.append(te)
                td = cpool.tile([P, TCORE], F32, tag=f"decT{k}", name=f"decT{k}")
                nc.sync.dma_start(td[:], decT[k * P : (k + 1) * P, :])
                decT_sb.append(td)
                tw = cpool.tile([P, A], F32, tag=f"wh{k}", name=f"wh{k}")
                nc.sync.dma_start(tw[:], wh[k * P : (k + 1) * P, :])
                wh_sb.append(tw)
                tw2 = cpool.tile([P, A], F32, tag=f"ws{k}", name=f"ws{k}")
                nc.sync.dma_start(tw2[:], ws[k * P : (k + 1) * P, :])
                ws_sb.append(tw2)
            bsum_sb = []
            wv_sb = []
            for j in range(JA):
                tb = cpool.tile([P, 1], F32, tag=f"bsum{j}", name=f"bsum{j}")
                nc.sync.dma_start(tb[:], bsum[j * P : (j + 1) * P, :])
                bsum_sb.append(tb)
                tv = cpool.tile([P, 1], F32, tag=f"wv{j}", name=f"wv{j}")
                nc.sync.dma_start(tv[:], wv[j * P : (j + 1) * P, :])
                wv_sb.append(tv)

            ehT = [
                cpool.tile([P, S], F32, tag=f"ehT{j}", name=f"ehT{j}")
                for j in range(JA)
            ]
            dh = [
                cpool.tile([P, TCORE], F32, tag=f"dh{j}", name=f"dh{j}")
                for j in range(JA)
            ]

            # Projections: ehT[j] = (Wh[:, j] block)^T @ encT, dh[j] likewise + bias.
            with tc.tile_pool(name="psum0", bufs=2, space="PSUM") as pp0:
                for j in range(JA):
                    for sh in range(NSH):
                        ps = pp0.tile([P, 512], F32, tag="ps0", name="ps0")
                        for k in range(KH):
                            nc.tensor.matmul(
                                ps[:],
                                wh_sb[k][:, j * P : (j + 1) * P],
                                encT_sb[k][:, sh * 512 : (sh + 1) * 512],
                                start=(k == 0),
                                stop=(k == KH - 1),
                            )
                        nc.vector.tensor_copy(
                            ehT[j][:, sh * 512 : (sh + 1) * 512], ps[:]
                        )
                for j in range(JA):
                    ps = pp0.tile([P, 512], F32, tag="ps0", name="ps0")
                    for k in range(KH):
                        nc.tensor.matmul(
                            ps[:, :TCORE],
                            ws_sb[k][:, j * P : (j + 1) * P],
                            decT_sb[k][:],
                            start=(k == 0),
                            stop=(k == KH - 1),
                        )
                    nc.vector.tensor_scalar_add(
                        dh[j][:], ps[:, :TCORE], bsum_sb[j][:]
                    )

            scores = cpool.tile([TCORE, S], F32, tag="scores", name="scores")

            # Engine SBUF access patterns must start at partition 0/32/64/96,
            # so score rows cannot be written straight to partition t.  Bounce
            # each [1, S] row through DRAM scratch; reload as [128, S] after.
            with (
                tc.tile_pool(name="tanhp", bufs=3) as tpool,
                tc.tile_pool(name="pscp", bufs=3, space="PSUM") as pscp,
                tc.tile_pool(name="rowp", bufs=4) as rowp,
                tc.tile_pool(name="dramp", bufs=1, space="DRAM") as dramp,
            ):
                scores_dram = dramp.tile(
                    [TCORE, S], F32, tag="scores_dram", name="scores_dram"
                )
                for t in range(TCORE):
                    psc = pscp.tile([1, S], F32, tag="psc", name="psc")
                    for j in range(JA):
                        th = tpool.tile([P, S], F32, tag=f"tanh{j}", name=f"tanh{j}")
                        nc.scalar.activation(
                            th[:],
                            ehT[j][:],
                            mybir.ActivationFunctionType.Tanh,
                            bias=dh[j][:, t : t + 1],
                        )
                        for sh in range(NSH):
                            nc.tensor.matmul(
                                psc[0:1, sh * 512 : (sh + 1) * 512],
                                wv_sb[j][:],
                                th[:, sh * 512 : (sh + 1) * 512],
                                start=(j == 0),
                                stop=(j == JA - 1),
                            )
                    srow = rowp.tile([1, S], F32, tag="srow", name="srow")
                    nc.vector.tensor_copy(srow[0:1, :], psc[0:1, :])
                    nc.sync.dma_start(scores_dram[t : t + 1, :], srow[0:1, :])
                nc.sync.dma_start(scores[:], scores_dram[:])

            # Softmax over s (free dim), batched over all 128 t-rows.
            with tc.tile_pool(name="soft", bufs=1) as spool:
                nmx = spool.tile([TCORE, 1], F32, tag="nmx", name="nmx")
                nc.vector.tensor_reduce(
                    nmx[:],
                    scores[:],
                    axis=mybir.AxisListType.X,
                    op=mybir.AluOpType.max,
                    negate=True,
                )
                probs = spool.tile([TCORE, S], F32, tag="probs", name="probs")
                nc.scalar.activation(
                    probs[:],
                    scores[:],
                    mybir.ActivationFunctionType.Exp,
                    bias=nmx[:],
                )
                sm = spool.tile([TCORE, 1], F32, tag="sm", name="sm")
                nc.vector.reduce_sum(sm[:], probs[:], axis=mybir.AxisListType.X)
                rc = spool.tile([TCORE, 1], F32, tag="rc", name="rc")
                nc.vector.reciprocal(rc[:], sm[:])
                outsb = spool.tile([TCORE, S], F32, tag="outsb", name="outsb")
                nc.vector.tensor_scalar_mul(outsb[:], probs[:], rc[:])
                nc.sync.dma_start(out[:], outsb[:])

    nc.finalize()
    return nc


def make_in_maps(
    enc: np.ndarray,
    dec: np.ndarray,
    Wh: np.ndarray,
    bh: np.ndarray,
    Ws: np.ndarray,
    bs: np.ndarray,
    Wv: np.ndarray,
) -> list[dict[str, np.ndarray]]:
    bsum = (bh + bs).reshape(A, 1).astype(np.float32)
    wv = Wv.reshape(A, 1).astype(np.float32)
    in_maps = []
    for c in range(NCORES):
        b = c // 2
        t0 = (c % 2) * TCORE
        in_maps.append(
            {
                "encT": np.ascontiguousarray(enc[b].T),
                "decT": np.ascontiguousarray(dec[b, t0 : t0 + TCORE].T),
                "wh": np.ascontiguousarray(Wh),
                "ws": np.ascontiguousarray(Ws),
                "bsum": bsum,
                "wv": wv,
            }
        )
    return in_maps


_NC_CACHE: bass.Bass | None = None


def _get_nc() -> bass.Bass:
    global _NC_CACHE
    if _NC_CACHE is None:
        _NC_CACHE = build_bass()
    return _NC_CACHE


def kernel(**inputs: np.ndarray) -> np.ndarray:
    enc = np.asarray(inputs["encoder_outputs"], dtype=np.float32)
    dec = np.asarray(inputs["decoder_hidden"], dtype=np.float32)
    Wh = np.asarray(inputs["Wh"], dtype=np.float32)
    bh = np.asarray(inputs["bh"], dtype=np.float32)
    Ws = np.asarray(inputs["Ws"], dtype=np.float32)
    bs = np.asarray(inputs["bs"], dtype=np.float32)
    Wv = np.asarray(inputs["Wv"], dtype=np.float32)

    nc = _get_nc()
    in_maps = make_in_maps(enc, dec, Wh, bh, Ws, bs, Wv)
    res = run_bass_kernel_spmd(nc, in_maps, list(range(NCORES)))
    outs = np.stack([res.results[c]["out"] for c in range(NCORES)])
    return outs.reshape(B, 2, TCORE, S).reshape(B, T, S)


if __name__ == "__main__":
    rng = np.random.default_rng(0)
    ins = {
        "encoder_outputs": rng.standard_normal((B, S, H), dtype=np.float32),
        "decoder_hidden": rng.standard_normal((B, T, H), dtype=np.float32),
        "Wh": rng.standard_normal((H, A), dtype=np.float32) / np.sqrt(H),
        "bh": rng.standard_normal((A,), dtype=np.float32) * 0.01,
        "Ws": rng.standard_normal((H, A), dtype=np.float32) / np.sqrt(H),
        "bs": rng.standard_normal((A,), dtype=np.float32) * 0.01,
        "Wv": rng.standard_normal((A, 1), dtype=np.float32) / np.sqrt(A),
        "bv": rng.standard_normal((1,), dtype=np.float32) * 0.01,
    }
    out = kernel(**ins)
    print("kernel out", out.shape, out.dtype, out.sum())


# revision 15
# speedup vs baseline: 259.7910x; 259.7910x over previous
"""Bahdanau additive attention on 8 trn2 NeuronCores.

Computation (per batch b):
    eh = enc[b] @ Wh + bh                    # [S, A]
    dh = dec[b] @ Ws + bs                    # [T, A]
    scores[t, s] = Wv . tanh(eh[s] + dh[t])  (+ bv, dropped: softmax-invariant)
    out[t, :] = softmax(scores[t, :])

Sharding: core c handles batch b = c//2 and decoder rows t in
[128*(c%2), 128*(c%2)+128).  Weights replicated; no cross-core comm.

Per-core kernel layout: A (=256) on partitions in two 128-chunks.
ScalarE activation computes tanh(ehT[a, s] + dhT[a, t]) with the
broadcast-add fused in via the per-partition bias operand.  The
weighted reduction over A is a TensorE matmul with lhsT = Wv [128, 1]
writing scores rows [1, S] into PSUM; softmax is batched on the
[128, 1024] scores tile at the end.
"""

import sys

import numpy as np

sys.path.insert(0, "/opt/trn_rl_repo")

import concourse.bass as bass
import concourse.bacc as bacc
import concourse.tile as tile
from concourse import mybir
from concourse.bass_utils import run_bass_kernel_spmd

B, S, T, H, A = 4, 1024, 256, 512, 256
NCORES = 8
TCORE = (B * T) // NCORES  # 128 decoder rows per core
F32 = mybir.dt.float32
F16 = mybir.dt.float16
P = 128
KH = H // P  # 4 contraction chunks for the projections
JA = A // P  # 2 partition chunks of the attention dim
NSH = S // 512  # 2 matmul free-dim slices of S


def build_bass(repeat: int = 1, G: int = 4) -> bass.Bass:
    """repeat > 1 wraps the whole body in an on-device loop — used only for
    wall-clock benchmarking (amplifies device time over RPC overhead)."""
    import contextlib

    nc = bacc.Bacc()
    encT = nc.declare_dram_parameter("encT", [H, S], F16, isOutput=False)
    decT = nc.declare_dram_parameter("decT", [H, TCORE], F16, isOutput=False)
    wh = nc.declare_dram_parameter("wh", [H, A], F16, isOutput=False)
    ws = nc.declare_dram_parameter("ws", [H, A], F16, isOutput=False)
    bsum = nc.declare_dram_parameter("bsum", [A, 1], F32, isOutput=False)
    wv = nc.declare_dram_parameter("wv", [A, 32], F16, isOutput=False)
    out = nc.declare_dram_parameter("out", [TCORE, S], F32, isOutput=True)

    with tile.TileContext(nc) as tc:
        rep_ctx = (
            tc.For_i(0, repeat, 1) if repeat > 1 else contextlib.nullcontext()
        )
        with rep_ctx, tc.tile_pool(name="const", bufs=1) as cpool:
            encT_sb = []
            decT_sb = []
            wh_sb = []
            ws_sb = []
            for k in range(KH):
                te = cpool.tile([P, S], F16, tag=f"encT{k}", name=f"encT{k}")
                nc.sync.dma_start(te[:], encT[k * P : (k + 1) * P, :])
                encT_sb.append(te)
                td = cpool.tile([P, TCORE], F16, tag=f"decT{k}", name=f"decT{k}")
                nc.sync.dma_start(td[:], decT[k * P : (k + 1) * P, :])
                decT_sb.append(td)
                tw = cpool.tile([P, A], F16, tag=f"wh{k}", name=f"wh{k}")
                nc.sync.dma_start(tw[:], wh[k * P : (k + 1) * P, :])
                wh_sb.append(tw)
                tw2 = cpool.tile([P, A], F16, tag=f"ws{k}", name=f"ws{k}")
                nc.sync.dma_start(tw2[:], ws[k * P : (k + 1) * P, :])
                ws_sb.append(tw2)
            bsum_sb = []
            wv_sb = []
            for j in range(JA):
                tb = cpool.tile([P, 1], F32, tag=f"bsum{j}", name=f"bsum{j}")
                nc.sync.dma_start(tb[:], bsum[j * P : (j + 1) * P, :])
                bsum_sb.append(tb)
                tv = cpool.tile([P, 32], F16, tag=f"wv{j}", name=f"wv{j}")
                nc.sync.dma_start(tv[:], wv[j * P : (j + 1) * P, :])
                wv_sb.append(tv)

            ehT = [
                cpool.tile([P, S], F16, tag=f"ehT{j}", name=f"ehT{j}")
                for j in range(JA)
            ]
            dh = [
                cpool.tile([P, TCORE], F32, tag=f"dh{j}", name=f"dh{j}")
                for j in range(JA)
            ]

            # Projections: ehT[j] = (Wh[:, j] block)^T @ encT, dh[j] likewise + bias.
            with tc.tile_pool(name="psum0", bufs=2, space="PSUM") as pp0:
                for j in range(JA):
                    for sh in range(NSH):
                        ps = pp0.tile([P, 512], F32, tag="ps0", name="ps0")
                        for k in range(KH):
                            nc.tensor.matmul(
                                ps[:],
                                wh_sb[k][:, j * P : (j + 1) * P],
                                encT_sb[k][:, sh * 512 : (sh + 1) * 512],
                                start=(k == 0),
                                stop=(k == KH - 1),
                            )
                        nc.vector.tensor_copy(
                            ehT[j][:, sh * 512 : (sh + 1) * 512], ps[:]
                        )
                for j in range(JA):
                    ps = pp0.tile([P, 512], F32, tag="ps0", name="ps0")
                    for k in range(KH):
                        nc.tensor.matmul(
                            ps[:, :TCORE],
                            ws_sb[k][:, j * P : (j + 1) * P],
                            decT_sb[k][:],
                            start=(k == 0),
                            stop=(k == KH - 1),
                        )
                    nc.vector.tensor_scalar_add(
                        dh[j][:], ps[:, :TCORE], bsum_sb[j][:]
                    )

            scores_c = [
                cpool.tile([TCORE // 2, S], F32, tag=f"scores{c}", name=f"scores{c}")
                for c in range(2)
            ]

            # Main loop.  tanh tiles are fp16 (fp32 matmuls cost 4 cyc/row on
            # PE; fp16 costs 1).  Wv comes in replicated to [A, 32] so each
            # matmul has M=32 and fills a whole 32-partition PSUM quadrant
            # (tile_position column groups); 4 t-rows land on partitions
            # {0,32,64,96} of one [128, S] psum tile.  One wide DVE copy
            # moves all 4 to SBUF, and a partition-strided DMA scatters them
            # to DRAM scratch (engines can't write partition t directly —
            # SBUF APs must start at partition 0/32/64/96).
            with (
                tc.tile_pool(name="tanhp", bufs=3) as tpool,
                tc.tile_pool(name="pscp", bufs=3, space="PSUM") as pscp,
                tc.tile_pool(name="rowp", bufs=4) as rowp,
                tc.tile_pool(name="dramp", bufs=1, space="DRAM") as dramp,
            ):
                scores_dram_c = [
                    dramp.tile(
                        [TCORE // 2, S],
                        F32,
                        tag=f"scores_dram{c}",
                        name=f"scores_dram{c}",
                    )
                    for c in range(2)
                ]
                # G = decoder rows per ACT instruction
                for r in range(TCORE // 4):
                    g, rr = divmod(r, G // 4)
                    if rr == 0:
                        # DVE pre-adds E = ehT + dh[t] for G rows (4x mode,
                        # fp16), then ONE in-place tanh over FD = G*S —
                        # amortizes the ~425-cycle ACT per-instr overhead.
                        th_g = []
                        for j in range(JA):
                            th = tpool.tile(
                                [P, G * S], F16, tag=f"tanh{j}", name=f"tanh{j}"
                            )
                            for u in range(G):
                                t = g * G + u
                                nc.vector.tensor_scalar_add(
                                    th[:, u * S : (u + 1) * S],
                                    ehT[j][:],
                                    dh[j][:, t : t + 1],
                                )
                            nc.scalar.activation(
                                th[:], th[:], mybir.ActivationFunctionType.Tanh
                            )
                            th_g.append(th)
                    psg = pscp.tile([P, S], F32, tag="psg", name="psg")
                    for q in range(4):
                        u = rr * 4 + q
                        for j in range(JA):
                            for sh in range(NSH):
                                nc.tensor.matmul(
                                    psg[
                                        32 * q : 32 * q + 32,
                                        sh * 512 : (sh + 1) * 512,
                                    ],
                                    wv_sb[j][:],
                                    th_g[j][
                                        :, u * S + sh * 512 : u * S + (sh + 1) * 512
                                    ],
                                    start=(j == 0),
                                    stop=(j == JA - 1),
                                    tile_position=(0, 32 * q),
                                )
                    gath = rowp.tile([P, S], F32, tag="gath", name="gath")
                    nc.vector.tensor_copy(gath[:], psg[:])
                    # rows {0,32,64,96} hold t = 4r+0..4r+3
                    gsel = gath.rearrange("(q w) f -> q w f", w=32)[:, 0, :]
                    rc_, ro = divmod(4 * r, TCORE // 2)
                    nc.sync.dma_start(
                        scores_dram_c[rc_][ro : ro + 4, :], gsel
                    )

                    # Softmax a 64-row half as soon as its rounds are done so
                    # the tail overlaps the remaining main loop.  All APs in
                    # the half start at partition 0 or 64 (engine-legal).
                    if (r + 1) % (TCORE // 8) == 0:
                        c = (r + 1) // (TCORE // 8) - 1
                        HC = TCORE // 2
                        sc = scores_c[c]
                        nc.sync.dma_start(sc[:], scores_dram_c[c][:])
                        nmx = rowp.tile(
                            [HC, 1], F32, tag=f"nmx{c}", name=f"nmx{c}", bufs=1
                        )
                        nc.vector.tensor_reduce(
                            nmx[:],
                            sc[:],
                            axis=mybir.AxisListType.X,
                            op=mybir.AluOpType.max,
                            negate=True,
                        )
                        probs = rowp.tile(
                            [HC, S], F32, tag=f"probs{c}", name=f"probs{c}", bufs=1
                        )
                        nc.scalar.activation(
                            probs[:],
                            sc[:],
                            mybir.ActivationFunctionType.Exp,
                            bias=nmx[:],
                        )
                        sm = rowp.tile(
                            [HC, 1], F32, tag=f"sm{c}", name=f"sm{c}", bufs=1
                        )
                        nc.vector.reduce_sum(
                            sm[:], probs[:], axis=mybir.AxisListType.X
                        )
                        rcp = rowp.tile(
                            [HC, 1], F32, tag=f"rc{c}", name=f"rc{c}", bufs=1
                        )
                        nc.vector.reciprocal(rcp[:], sm[:])
                        outsb = rowp.tile(
                            [HC, S], F32, tag=f"outsb{c}", name=f"outsb{c}", bufs=1
                        )
                        nc.vector.tensor_scalar_mul(
                            outsb[:], probs[:], rcp[:]
                        )
                        nc.sync.dma_start(
                            out[HC * c : HC * (c + 1), :], outsb[:]
                        )

    nc.finalize()
    return nc


def make_in_maps(
    enc: np.ndarray,
    dec: np.ndarray,
    Wh: np.ndarray,
    bh: np.ndarray,
    Ws: np.ndarray,
    bs: np.ndarray,
    Wv: np.ndarray,
) -> list[dict[str, np.ndarray]]:
    bsum = (bh + bs).reshape(A, 1).astype(np.float32)
    wv = np.ascontiguousarray(
        np.broadcast_to(Wv.reshape(A, 1), (A, 32))
    ).astype(np.float16)
    in_maps = []
    for c in range(NCORES):
        b = c // 2
        t0 = (c % 2) * TCORE
        in_maps.append(
            {
                "encT": np.ascontiguousarray(enc[b].T).astype(np.float16),
                "decT": np.ascontiguousarray(dec[b, t0 : t0 + TCORE].T).astype(
                    np.float16
                ),
                "wh": np.ascontiguousarray(Wh).astype(np.float16),
                "ws": np.ascontiguousarray(Ws).astype(np.float16),
                "bsum": bsum,
                "wv": wv,
            }
        )
    return in_maps


_NC_CACHE: bass.Bass | None = None


def _get_nc() -> bass.Bass:
    global _NC_CACHE
    if _NC_CACHE is None:
        _NC_CACHE = build_bass()
    return _NC_CACHE


def kernel(**inputs: np.ndarray) -> np.ndarray:
    enc = np.asarray(inputs["encoder_outputs"], dtype=np.float32)
    dec = np.asarray(inputs["decoder_hidden"], dtype=np.float32)
    Wh = np.asarray(inputs["Wh"], dtype=np.float32)
    bh = np.asarray(inputs["bh"], dtype=np.float32)
    Ws = np.asarray(inputs["Ws"], dtype=np.float32)
    bs = np.asarray(inputs["bs"], dtype=np.float32)
    Wv = np.asarray(inputs["Wv"], dtype=np.float32)

    nc = _get_nc()
    in_maps = make_in_maps(enc, dec, Wh, bh, Ws, bs, Wv)
    res = run_bass_kernel_spmd(nc, in_maps, list(range(NCORES)))
    outs = np.stack([res.results[c]["out"] for c in range(NCORES)])
    return outs.reshape(B, 2, TCORE, S).reshape(B, T, S)


if __name__ == "__main__":
    rng = np.random.default_rng(0)
    ins = {
        "encoder_outputs": rng.standard_normal((B, S, H), dtype=np.float32),
        "decoder_hidden": rng.standard_normal((B, T, H), dtype=np.float32),
        "Wh": rng.standard_normal((H, A), dtype=np.float32) / np.sqrt(H),
        "bh": rng.standard_normal((A,), dtype=np.float32) * 0.01,
        "Ws": rng.standard_normal((H, A), dtype=np.float32) / np.sqrt(H),
        "bs": rng.standard_normal((A,), dtype=np.float32) * 0.01,
        "Wv": rng.standard_normal((A, 1), dtype=np.float32) / np.sqrt(A),
        "bv": rng.standard_normal((1,), dtype=np.float32) * 0.01,
    }
    out = kernel(**ins)
    print("kernel out", out.shape, out.dtype, out.sum())


# revision 18
# speedup vs baseline: 262.6290x; 1.0109x over previous
"""Bahdanau additive attention on 8 trn2 NeuronCores.

Computation (per batch b):
    eh = enc[b] @ Wh + bh                    # [S, A]
    dh = dec[b] @ Ws + bs                    # [T, A]
    scores[t, s] = Wv . tanh(eh[s] + dh[t])  (+ bv, dropped: softmax-invariant)
    out[t, :] = softmax(scores[t, :])

Sharding: core c handles batch b = c//2 and decoder rows t in
[128*(c%2), 128*(c%2)+128).  Weights replicated; no cross-core comm.

Per-core kernel layout: A (=256) on partitions in two 128-chunks.
ScalarE activation computes tanh(ehT[a, s] + dhT[a, t]) with the
broadcast-add fused via the per-partition bias operand (ScalarE is the
bottleneck engine: ~33.5M tanh/core at ~1.4 cyc/elem; measured kernel
time equals the bare tanh-stream time, i.e. all other engines hide).
The weighted reduction over A is a TensorE matmul with fp16 operands
(fp32 would cost 4 cyc/row) and lhsT = Wv replicated to [128, 32], so
M=32 fills a whole 32-partition PSUM quadrant per tile_position column
group — 4 t-rows per [128, S] psum tile, one wide DVE copy out, and a
partition-strided DMA to DRAM scratch (engine SBUF APs must start at
partition 0/32/64/96, so rows can't be scattered to partition t
directly).  Each 64-row half is softmaxed as soon as its rounds finish
so the tail overlaps the main loop.
"""

import sys

import numpy as np

sys.path.insert(0, "/opt/trn_rl_repo")

import concourse.bass as bass
import concourse.bacc as bacc
import concourse.tile as tile
from concourse import mybir
from concourse.bass_utils import run_bass_kernel_spmd

B, S, T, H, A = 4, 1024, 256, 512, 256
NCORES = 8
TCORE = (B * T) // NCORES  # 128 decoder rows per core
F32 = mybir.dt.float32
F16 = mybir.dt.float16
P = 128
KH = H // P  # 4 contraction chunks for the projections
JA = A // P  # 2 partition chunks of the attention dim
NSH = S // 512  # 2 matmul free-dim slices of S


def build_bass(repeat: int = 1, G: int = 1) -> bass.Bass:
    """repeat > 1 wraps the whole body in an on-device loop — used only for
    wall-clock benchmarking (amplifies device time over RPC overhead)."""
    import contextlib

    nc = bacc.Bacc()
    encT = nc.declare_dram_parameter("encT", [H, S], F16, isOutput=False)
    decT = nc.declare_dram_parameter("decT", [H, TCORE], F16, isOutput=False)
    wh = nc.declare_dram_parameter("wh", [H, A], F16, isOutput=False)
    ws = nc.declare_dram_parameter("ws", [H, A], F16, isOutput=False)
    bsum = nc.declare_dram_parameter("bsum", [A, 1], F32, isOutput=False)
    wv = nc.declare_dram_parameter("wv", [A, 32], F16, isOutput=False)
    out = nc.declare_dram_parameter("out", [TCORE, S], F32, isOutput=True)

    with tile.TileContext(nc) as tc:
        rep_ctx = (
            tc.For_i(0, repeat, 1) if repeat > 1 else contextlib.nullcontext()
        )
        with rep_ctx, tc.tile_pool(name="const", bufs=1) as cpool:
            encT_sb = []
            decT_sb = []
            wh_sb = []
            ws_sb = []
            for k in range(KH):
                te = cpool.tile([P, S], F16, tag=f"encT{k}", name=f"encT{k}")
                nc.sync.dma_start(te[:], encT[k * P : (k + 1) * P, :])
                encT_sb.append(te)
                td = cpool.tile([P, TCORE], F16, tag=f"decT{k}", name=f"decT{k}")
                nc.sync.dma_start(td[:], decT[k * P : (k + 1) * P, :])
                decT_sb.append(td)
                tw = cpool.tile([P, A], F16, tag=f"wh{k}", name=f"wh{k}")
                nc.sync.dma_start(tw[:], wh[k * P : (k + 1) * P, :])
                wh_sb.append(tw)
                tw2 = cpool.tile([P, A], F16, tag=f"ws{k}", name=f"ws{k}")
                nc.sync.dma_start(tw2[:], ws[k * P : (k + 1) * P, :])
                ws_sb.append(tw2)
            bsum_sb = []
            wv_sb = []
            for j in range(JA):
                tb = cpool.tile([P, 1], F32, tag=f"bsum{j}", name=f"bsum{j}")
                nc.sync.dma_start(tb[:], bsum[j * P : (j + 1) * P, :])
                bsum_sb.append(tb)
                tv = cpool.tile([P, 32], F16, tag=f"wv{j}", name=f"wv{j}")
                nc.sync.dma_start(tv[:], wv[j * P : (j + 1) * P, :])
                wv_sb.append(tv)

            ehT = [
                cpool.tile([P, S], F16, tag=f"ehT{j}", name=f"ehT{j}")
                for j in range(JA)
            ]
            dh = [
                cpool.tile([P, TCORE], F32, tag=f"dh{j}", name=f"dh{j}")
                for j in range(JA)
            ]

            # Projections: ehT[j] = (Wh[:, j] block)^T @ encT, dh[j] likewise + bias.
            with tc.tile_pool(name="psum0", bufs=2, space="PSUM") as pp0:
                for j in range(JA):
                    for sh in range(NSH):
                        ps = pp0.tile([P, 512], F32, tag="ps0", name="ps0")
                        for k in range(KH):
                            nc.tensor.matmul(
                                ps[:],
                                wh_sb[k][:, j * P : (j + 1) * P],
                                encT_sb[k][:, sh * 512 : (sh + 1) * 512],
                                start=(k == 0),
                                stop=(k == KH - 1),
                            )
                        nc.vector.tensor_copy(
                            ehT[j][:, sh * 512 : (sh + 1) * 512], ps[:]
                        )
                for j in range(JA):
                    ps = pp0.tile([P, 512], F32, tag="ps0", name="ps0")
                    for k in range(KH):
                        nc.tensor.matmul(
                            ps[:, :TCORE],
                            ws_sb[k][:, j * P : (j + 1) * P],
                            decT_sb[k][:],
                            start=(k == 0),
                            stop=(k == KH - 1),
                        )
                    nc.vector.tensor_scalar_add(
                        dh[j][:], ps[:, :TCORE], bsum_sb[j][:]
                    )

            scores_c = [
                cpool.tile([TCORE // 2, S], F32, tag=f"scores{c}", name=f"scores{c}")
                for c in range(2)
            ]

            # Main loop.  tanh tiles are fp16 (fp32 matmuls cost 4 cyc/row on
            # PE; fp16 costs 1).  Wv comes in replicated to [A, 32] so each
            # matmul has M=32 and fills a whole 32-partition PSUM quadrant
            # (tile_position column groups); 4 t-rows land on partitions
            # {0,32,64,96} of one [128, S] psum tile.  One wide DVE copy
            # moves all 4 to SBUF, and a partition-strided DMA scatters them
            # to DRAM scratch (engines can't write partition t directly —
            # SBUF APs must start at partition 0/32/64/96).
            with (
                tc.tile_pool(name="tanhp", bufs=3) as tpool,
                tc.tile_pool(name="pscp", bufs=3, space="PSUM") as pscp,
                tc.tile_pool(name="rowp", bufs=4) as rowp,
                tc.tile_pool(name="dramp", bufs=1, space="DRAM") as dramp,
            ):
                scores_dram_c = [
                    dramp.tile(
                        [TCORE // 2, S],
                        F32,
                        tag=f"scores_dram{c}",
                        name=f"scores_dram{c}",
                    )
                    for c in range(2)
                ]
                # G = decoder rows per ACT instruction
                for r in range(TCORE // 4):
                    g, rr = divmod(r, max(G // 4, 1))
                    if rr == 0 and G == 1:
                        # fused path: per-t ACT with bias, no DVE pre-add
                        th_g = []
                        for j in range(JA):
                            th = tpool.tile(
                                [P, 4 * S], F16, tag=f"tanh{j}", name=f"tanh{j}"
                            )
                            for u in range(4):
                                t = 4 * r + u
                                nc.scalar.activation(
                                    th[:, u * S : (u + 1) * S],
                                    ehT[j][:],
                                    mybir.ActivationFunctionType.Tanh,
                                    bias=dh[j][:, t : t + 1],
                                )
                            th_g.append(th)
                    elif rr == 0:
                        # DVE pre-adds E = ehT + dh[t] for G rows (4x mode,
                        # fp16), then ONE in-place tanh over FD = G*S —
                        # amortizes the ~425-cycle ACT per-instr overhead.
                        th_g = []
                        for j in range(JA):
                            th = tpool.tile(
                                [P, G * S], F16, tag=f"tanh{j}", name=f"tanh{j}"
                            )
                            for u in range(G):
                                t = g * G + u
                                nc.vector.tensor_scalar_add(
                                    th[:, u * S : (u + 1) * S],
                                    ehT[j][:],
                                    dh[j][:, t : t + 1],
                                )
                            nc.scalar.activation(
                                th[:], th[:], mybir.ActivationFunctionType.Tanh
                            )
                            th_g.append(th)
                    psg = pscp.tile([P, S], F32, tag="psg", name="psg")
                    for q in range(4):
                        u = (rr * 4 + q) if G > 1 else q
                        for j in range(JA):
                            for sh in range(NSH):
                                nc.tensor.matmul(
                                    psg[
                                        32 * q : 32 * q + 32,
                                        sh * 512 : (sh + 1) * 512,
                                    ],
                                    wv_sb[j][:],
                                    th_g[j][
                                        :, u * S + sh * 512 : u * S + (sh + 1) * 512
                                    ],
                                    start=(j == 0),
                                    stop=(j == JA - 1),
                                    tile_position=(0, 32 * q),
                                )
                    gath = rowp.tile([P, S], F32, tag="gath", name="gath")
                    nc.vector.tensor_copy(gath[:], psg[:])
                    # rows {0,32,64,96} hold t = 4r+0..4r+3
                    gsel = gath.rearrange("(q w) f -> q w f", w=32)[:, 0, :]
                    rc_, ro = divmod(4 * r, TCORE // 2)
                    nc.sync.dma_start(
                        scores_dram_c[rc_][ro : ro + 4, :], gsel
                    )

                    # Softmax a 64-row half as soon as its rounds are done so
                    # the tail overlaps the remaining main loop.  All APs in
                    # the half start at partition 0 or 64 (engine-legal).
                    if (r + 1) % (TCORE // 8) == 0:
                        c = (r + 1) // (TCORE // 8) - 1
                        HC = TCORE // 2
                        sc = scores_c[c]
                        nc.sync.dma_start(sc[:], scores_dram_c[c][:])
                        nmx = rowp.tile(
                            [HC, 1], F32, tag=f"nmx{c}", name=f"nmx{c}", bufs=1
                        )
                        nc.vector.tensor_reduce(
                            nmx[:],
                            sc[:],
                            axis=mybir.AxisListType.X,
                            op=mybir.AluOpType.max,
                            negate=True,
                        )
                        probs = rowp.tile(
                            [HC, S], F32, tag=f"probs{c}", name=f"probs{c}", bufs=1
                        )
                        nc.scalar.activation(
                            probs[:],
                            sc[:],
                            mybir.ActivationFunctionType.Exp,
                            bias=nmx[:],
                        )
                        sm = rowp.tile(
                            [HC, 1], F32, tag=f"sm{c}", name=f"sm{c}", bufs=1
                        )
                        nc.vector.reduce_sum(
                            sm[:], probs[:], axis=mybir.AxisListType.X
                        )
                        rcp = rowp.tile(
                            [HC, 1], F32, tag=f"rc{c}", name=f"rc{c}", bufs=1
                        )
                        nc.vector.reciprocal(rcp[:], sm[:])
                        outsb = rowp.tile(
                            [HC, S], F32, tag=f"outsb{c}", name=f"outsb{c}", bufs=1
                        )
                        nc.vector.tensor_scalar_mul(
                            outsb[:], probs[:], rcp[:]
                        )
                        nc.sync.dma_start(
                            out[HC * c : HC * (c + 1), :], outsb[:]
                        )

    nc.finalize()
    return nc


def make_in_maps(
    enc: np.ndarray,
    dec: np.ndarray,
    Wh: np.ndarray,
    bh: np.ndarray,
    Ws: np.ndarray,
    bs: np.ndarray,
    Wv: np.ndarray,
) -> list[dict[str, np.ndarray]]:
    bsum = (bh + bs).reshape(A, 1).astype(np.float32)
    wv = np.ascontiguousarray(
        np.broadcast_to(Wv.reshape(A, 1), (A, 32))
    ).astype(np.float16)
    in_maps = []
    for c in range(NCORES):
        b = c // 2
        t0 = (c % 2) * TCORE
        in_maps.append(
            {
                "encT": np.ascontiguousarray(enc[b].T).astype(np.float16),
                "decT": np.ascontiguousarray(dec[b, t0 : t0 + TCORE].T).astype(
                    np.float16
                ),
                "wh": np.ascontiguousarray(Wh).astype(np.float16),
                "ws": np.ascontiguousarray(Ws).astype(np.float16),
                "bsum": bsum,
                "wv": wv,
            }
        )
    return in_maps


_NC_CACHE: bass.Bass | None = None


def _get_nc() -> bass.Bass:
    global _NC_CACHE
    if _NC_CACHE is None:
        _NC_CACHE = build_bass()
    return _NC_CACHE


def kernel(**inputs: np.ndarray) -> np.ndarray:
    enc = np.asarray(inputs["encoder_outputs"], dtype=np.float32)
    dec = np.asarray(inputs["decoder_hidden"], dtype=np.float32)
    Wh = np.asarray(inputs["Wh"], dtype=np.float32)
    bh = np.asarray(inputs["bh"], dtype=np.float32)
    Ws = np.asarray(inputs["Ws"], dtype=np.float32)
    bs = np.asarray(inputs["bs"], dtype=np.float32)
    Wv = np.asarray(inputs["Wv"], dtype=np.float32)

    nc = _get_nc()
    in_maps = make_in_maps(enc, dec, Wh, bh, Ws, bs, Wv)
    res = run_bass_kernel_spmd(nc, in_maps, list(range(NCORES)))
    outs = np.stack([res.results[c]["out"] for c in range(NCORES)])
    return outs.reshape(B, 2, TCORE, S).reshape(B, T, S)


if __name__ == "__main__":
    rng = np.random.default_rng(0)
    ins = {
        "encoder_outputs": rng.standard_normal((B, S, H), dtype=np.float32),
        "decoder_hidden": rng.standard_normal((B, T, H), dtype=np.float32),
        "Wh": rng.standard_normal((H, A), dtype=np.float32) / np.sqrt(H),
        "bh": rng.standard_normal((A,), dtype=np.float32) * 0.01,
        "Ws": rng.standard_normal((H, A), dtype=np.float32) / np.sqrt(H),
        "bs": rng.standard_normal((A,), dtype=np.float32) * 0.01,
        "Wv": rng.standard_normal((A, 1), dtype=np.float32) / np.sqrt(A),
        "bv": rng.standard_normal((1,), dtype=np.float32) * 0.01,
    }
    out = kernel(**ins)
    print("kernel out", out.shape, out.dtype, out.sum())
